# revision 26
# baseline (speedup 1.0000x reference)
"""Trainium2 Bass kernel for nn_DenseBlockEnd (gnn_message_passing).

Computes, for each graph b (B=512, MAX_ATOM=256, F=256):
    out[b] = relu(mask[b] * (node[b] + sum_l beta1*A_l[b] @ W_in[l]
                                     + beta2*BO[b] @ W_out[0]))
with mask[b, m] = (m < mol_slice[b]).

Strategy: data-parallel over the batch axis, 64 graphs per NeuronCore on 8
cores.  The three layer matmuls are fused into a single K=768 accumulation
against host-premultiplied (beta * W) weight chunks.  Activations are cast
f32->bf16 during the HBM->SBUF DMA, transposed on the TensorEngine (so the
contraction dim lands on partitions), then matmul'd in bf16 with f32 PSUM
accumulation.  node_features are added in f32 on the VectorEngine and the
row mask + relu are applied on the ScalarEngine via a per-partition scale.
"""

import numpy as np
import ml_dtypes
from contextlib import ExitStack

import concourse.bass as bass
import concourse.tile as tile
from concourse import bacc, mybir
from concourse import bass_utils

B, M, F = 512, 256, 256
NCORES = 8
BS = B // NCORES          # graphs per core
G = 4                     # graphs per pipeline batch
NB = BS // G              # pipeline batches
NSLAB = 3                 # inblock_acts[0], inblock_acts[1], block_outputs[0]
P = 128

F32 = mybir.dt.float32
BF16 = mybir.dt.bfloat16

_cached_nc = None


def _build_nc():
    nc = bacc.Bacc(trn_type="TRN2", target_bir_lowering=False, debug=False)

    node_d = nc.dram_tensor("node", [BS, M, F], F32, kind="ExternalInput").ap()
    a0_d = nc.dram_tensor("a0", [BS, M, F], F32, kind="ExternalInput").ap()
    a1_d = nc.dram_tensor("a1", [BS, M, F], F32, kind="ExternalInput").ap()
    bo_d = nc.dram_tensor("bo", [BS, M, F], F32, kind="ExternalInput").ap()
    wc_d = nc.dram_tensor("wc", [2 * NSLAB, P, F], BF16, kind="ExternalInput").ap()
    mask_d = nc.dram_tensor("maskt", [2, P, BS], F32, kind="ExternalInput").ap()
    ident_d = nc.dram_tensor("ident", [P, P], BF16, kind="ExternalInput").ap()
    out_d = nc.dram_tensor("out", [BS, M, F], F32, kind="ExternalOutput").ap()

    slabs_d = (a0_d, a1_d, bo_d)

    with tile.TileContext(nc) as tc, ExitStack() as ctx:
        const_pool = ctx.enter_context(tc.tile_pool(name="const", bufs=1))
        raw_pool = ctx.enter_context(tc.tile_pool(name="raw", bufs=4))
        at_pool = ctx.enter_context(tc.tile_pool(name="at", bufs=30))
        out_pool = ctx.enter_context(tc.tile_pool(name="outp", bufs=3))
        psum_t_pool = ctx.enter_context(
            tc.tile_pool(name="psum_t", bufs=3, space="PSUM")
        )
        psum_o_pool = ctx.enter_context(
            tc.tile_pool(name="psum_o", bufs=5, space="PSUM")
        )

        # Constants: combined weights [f_chunk, o], row masks, identity.
        w_sb = const_pool.tile([P, 2 * NSLAB, F], BF16, name="w_sb")
        nc.sync.dma_start(w_sb[:], wc_d.rearrange("c p o -> p c o"))
        mask_sb = const_pool.tile([P, 2, BS], F32, name="mask_sb")
        nc.sync.dma_start(mask_sb[:], mask_d.rearrange("t p g -> p t g"))
        ident_sb = const_pool.tile([P, P], BF16, name="ident_sb")
        nc.sync.dma_start(ident_sb[:], ident_d[:])

        # Atom rows are packed two-per-partition (m = 2p + j, j inner) so every
        # DMA descriptor covers 2 contiguous DRAM rows (2 KB) instead of 1.
        evac_parity = 0
        for bi in range(NB):
            g0 = bi * G
            # ---- loads ----
            node_raw = raw_pool.tile([P, G, 2, F], F32, name="node_raw", tag="node")
            nc.sync.dma_start(
                node_raw[:],
                node_d[g0 : g0 + G].rearrange("g (p j) f -> p g j f", j=2),
            )
            a_raws = []
            for s in range(NSLAB):
                a_raw = raw_pool.tile(
                    [P, G, 2, F], BF16, name=f"a{s}_raw", tag=f"a{s}"
                )
                # SWDGE DMA with f32 -> bf16 cast in flight.  The first batch
                # loads per-graph so the PE pipeline starts ASAP.
                if bi == 0:
                    for gi in range(G):
                        nc.gpsimd.dma_start(
                            a_raw[:, gi : gi + 1],
                            slabs_d[s][g0 + gi : g0 + gi + 1].rearrange(
                                "g (p j) f -> p g j f", j=2
                            ),
                        )
                else:
                    nc.gpsimd.dma_start(
                        a_raw[:],
                        slabs_d[s][g0 : g0 + G].rearrange("g (p j) f -> p g j f", j=2),
                    )
                a_raws.append(a_raw)

            out_sb = out_pool.tile([P, G, 2, F], F32, name="out_sb", tag="out")

            for gi in range(G):
                # ---- transpose A slabs: [m, f] -> [f, m] via PE ----
                ats = []
                for s in range(NSLAB):
                    psum_t = psum_t_pool.tile(
                        [P, 2, F], BF16, name=f"psum_t{s}", tag="pt"
                    )
                    for j in range(2):
                        for fc in range(2):
                            nc.tensor.transpose(
                                psum_t[:, fc, j * P : (j + 1) * P],
                                a_raws[s][:, gi, j, fc * P : (fc + 1) * P],
                                ident_sb[:],
                            )
                    at = at_pool.tile([P, 2, F], BF16, name=f"at{s}", tag="at")
                    nc.vector.tensor_copy(at[:], psum_t[:])
                    ats.append(at)

                # ---- matmuls: psum_o[m, o] = sum_s,fc A_s^T(fc, m)^T @ W(s, fc) ----
                psum_o = psum_o_pool.tile([P, 2, F], F32, name="psum_o", tag="po")
                for j in range(2):
                    first = True
                    for s in range(NSLAB):
                        for fc in range(2):
                            nc.tensor.matmul(
                                psum_o[:, j, :],
                                ats[s][:, fc, j * P : (j + 1) * P],
                                w_sb[:, 2 * s + fc, :],
                                start=first,
                                stop=(s == NSLAB - 1 and fc == 1),
                            )
                            first = False

                # ---- epilogue: add node (f32), then relu(mask * x) ----
                for j in range(2):
                    nc.vector.tensor_add(
                        out_sb[:, gi, j, :],
                        psum_o[:, j, :],
                        node_raw[:, gi, j, :],
                    )
                    nc.scalar.activation(
                        out_sb[:, gi, j, :],
                        out_sb[:, gi, j, :],
                        mybir.ActivationFunctionType.Relu,
                        scale=mask_sb[:, j, g0 + gi : g0 + gi + 1],
                    )

                # Store per graph so the tail drains as soon as each graph is done.
                nc.scalar.dma_start(
                    out_d[g0 + gi : g0 + gi + 1].rearrange(
                        "g (p j) f -> p g j f", j=2
                    ),
                    out_sb[:, gi : gi + 1],
                )

    nc.compile()
    return nc


def _in_maps(node, inb, bo, wc, maskt_all, ident):
    maps = []
    for c in range(NCORES):
        sl = slice(c * BS, (c + 1) * BS)
        maps.append(
            {
                "node": np.ascontiguousarray(node[sl]),
                "a0": np.ascontiguousarray(inb[0, sl]),
                "a1": np.ascontiguousarray(inb[1, sl]),
                "bo": np.ascontiguousarray(bo[0, sl]),
                "wc": wc,
                "maskt": maskt_all[c],
                "ident": ident,
            }
        )
    return maps


def _prep_in_maps(
    node_features,
    inblock_acts,
    block_outputs,
    mol_slice,
    W_in,
    W_out,
    beta1,
    beta2,
):
    node = np.asarray(node_features, dtype=np.float32)
    inb = np.asarray(inblock_acts, dtype=np.float32)
    bo = np.asarray(block_outputs, dtype=np.float32)
    mol = np.asarray(mol_slice, dtype=np.int32)
    w_in = np.asarray(W_in, dtype=np.float32)
    w_out = np.asarray(W_out, dtype=np.float32)
    b1 = float(np.asarray(beta1).reshape(-1)[0])
    b2 = float(np.asarray(beta2).reshape(-1)[0])

    wc = (
        np.concatenate([b1 * w_in[0], b1 * w_in[1], b2 * w_out[0]], axis=0)
        .reshape(2 * NSLAB, P, F)
        .astype(ml_dtypes.bfloat16)
    )
    mask = (np.arange(M)[None, :] < mol[:, None]).astype(np.float32)  # [B, M]
    # maskt[j, p, g] = mask[g, 2p + j] (row-pair packing, j inner)
    maskt_all = [
        np.ascontiguousarray(
            mask[c * BS : (c + 1) * BS].reshape(BS, P, 2).transpose(2, 1, 0)
        )
        for c in range(NCORES)
    ]
    ident = np.eye(P, dtype=ml_dtypes.bfloat16)
    return _in_maps(node, inb, bo, wc, maskt_all, ident)


def get_nc():
    global _cached_nc
    if _cached_nc is None:
        _cached_nc = _build_nc()
    return _cached_nc


def kernel(**inputs):
    nc = get_nc()
    res = bass_utils.run_bass_kernel_spmd(
        nc, _prep_in_maps(**inputs), core_ids=list(range(NCORES))
    )
    return np.concatenate([res.results[c]["out"] for c in range(NCORES)], axis=0)
